# revision 29
# baseline (speedup 1.0000x reference)
"""Multi-head attention (B=2, N=2048, D=1024, H=16, hd=64) on 8 trn2 NeuronCores.

Sharding: 8 cores = 2 (batch) x 4 (head groups of 4 heads).
Core c: batch b = c // 4, heads hg*4 .. hg*4+3 where hg = c % 4.

Per-core program (identical SPMD program, per-core data):
  inputs (DRAM):
    xT     [1024, 2048]  = x[b].T
    wqkT   [1024, 512]   = w_qkv[[q rows, k rows] of local heads].T
    wvT    [1024, 256]   = w_qkv[v rows of local heads].T
    wprojT [256, 1024]   = w_proj[:, local head cols].T
  output:
    out    [2048, 1024]  bf16 partial (row-parallel) projection output

Pipeline (v2): attention runs as 8 chains (pair-minor: 4 q-blocks of the
head pair (0,1), then 4 of pair (2,3)).  Per chain, per key tile kt:
  - 2 score MMs (heads at PE row groups 0/64 -> concurrent), one
    [128,1024] sc PSUM tile
  - 1 ACT exp over both heads' scores -> probs bf16 in SBUF
  - 2 PV MMs col-packed (M=64 at col groups 0/64) into one pv bank
  - 2 denominator MMs (ones lhsT, M=1 at partitions 0/32) into den bank
ACT is the bottleneck engine (~147us of exp); qkv gemms, v gemm, proj
and output DMA are emitted as paced "filler" units inside the chains so
the PE works in the gaps of the ACT-paced pipeline.  DMAs are emitted
nb-major so the first chain chases the x DMA.

Normalize tail per chain: reciprocal_approx_fast of the two denom rows,
block-ones broadcast matmul, DVE multiply straight into ao_sb (proj lhsT
layout).  Host unshard: out[b] = sum over 4 head-group partials + b_proj.
"""

import sys

if "/opt/trn_rl_repo" not in sys.path:
    sys.path.insert(0, "/opt/trn_rl_repo")

import numpy as np

B, N, D, H, HD = 2, 2048, 1024, 16, 64
NCORES = 8
HPC = 4               # heads per core
LQK = HPC * HD        # 256 local q (or k) rows
SCALE = HD ** -0.5    # 0.125

_CACHE = {}


def _build_program(debug=False):
    import concourse.tile as tile
    from concourse import bacc, mybir

    F32 = mybir.dt.float32
    F32R = mybir.dt.float32r
    BF16 = mybir.dt.bfloat16
    Exp = mybir.ActivationFunctionType.Exp

    nc = bacc.Bacc("TRN2", target_bir_lowering=False, debug=False,
                   num_devices=NCORES)

    KT = D // 128        # 8 contraction tiles for qkv gemms
    NB = N // 512        # 4 seq blocks
    NT = N // 128        # 16 seq tiles

    xT_d = nc.dram_tensor("xT", [D, N], BF16, kind="ExternalInput").ap()
    wqkT_d = nc.dram_tensor("wqkT", [D, 2 * LQK], BF16, kind="ExternalInput").ap()
    wvT_d = nc.dram_tensor("wvT", [D, LQK], BF16, kind="ExternalInput").ap()
    wprojT_d = nc.dram_tensor("wprojT", [LQK, D], BF16, kind="ExternalInput").ap()
    out_d = nc.dram_tensor("out", [N, D], BF16, kind="ExternalOutput").ap()
    if debug:
        dbg_qk_d = nc.dram_tensor("dbg_qk", [128, 4 * N], BF16,
                                  kind="ExternalOutput").ap()
        dbg_v_d = nc.dram_tensor("dbg_v", [128, NT * HPC * HD], BF16,
                                 kind="ExternalOutput").ap()
        dbg_ao_d = nc.dram_tensor("dbg_ao", [128, 2 * N], BF16,
                                  kind="ExternalOutput").ap()
        dbg_den_d = nc.dram_tensor("dbg_den", [128, 512], mybir.dt.float32,
                                   kind="ExternalOutput").ap()
        dbg_pv_d = nc.dram_tensor("dbg_pv", [128, 512], mybir.dt.float32,
                                  kind="ExternalOutput").ap()
        dbg_recip_d = nc.dram_tensor("dbg_recip", [1, 1024], mybir.dt.float32,
                                     kind="ExternalOutput").ap()
        dbg_bcs_d = nc.dram_tensor("dbg_bcs", [128, 512], mybir.dt.float32,
                                   kind="ExternalOutput").ap()
        dbg_sc_d = nc.dram_tensor("dbg_sc", [128, 1024], mybir.dt.float32,
                                  kind="ExternalOutput").ap()
        dbg_pr_d = nc.dram_tensor("dbg_pr", [128, 1024], BF16,
                                  kind="ExternalOutput").ap()

    xT_r = xT_d.rearrange("(kt p) n -> p kt n", p=128)
    wqkT_r = wqkT_d.rearrange("(kt p) m -> p kt m", p=128)
    wvT_r = wvT_d.rearrange("(kt p) m -> p kt m", p=128)
    wprojT_r = wprojT_d.rearrange("(kt p) o -> p kt o", p=128)

    with tile.TileContext(nc) as tc:
        with (
            nc.allow_low_precision(reason="bf16/f32r matmul operands"),
            tc.tile_pool(name="const", bufs=1) as cpool,
            tc.tile_pool(name="w", bufs=1) as wpool,
            tc.tile_pool(name="x", bufs=1) as xpool,
            tc.tile_pool(name="qk", bufs=1) as qkpool,
            tc.tile_pool(name="v", bufs=1) as vpool,
            tc.tile_pool(name="ao", bufs=1) as aopool,
            tc.tile_pool(name="probs", bufs=3) as prpool,
            tc.tile_pool(name="small", bufs=2) as smpool,
            tc.tile_pool(name="outs", bufs=2) as outpool,
            tc.tile_pool(name="pssc", bufs=2, space="PSUM") as pssc,
            tc.tile_pool(name="pspv", bufs=1, space="PSUM") as pspv,
            tc.tile_pool(name="psden", bufs=1, space="PSUM") as psden,
            tc.tile_pool(name="psprj", bufs=2, space="PSUM") as psprj,
        ):
            # ---- constants ----
            ones_f32 = cpool.tile([128, 128], F32)
            nc.vector.memset(ones_f32[:, :], 1.0)
            ones_col = cpool.tile([128, 1], BF16)
            nc.vector.tensor_copy(ones_col[:, :], ones_f32[:, 0:1])
            # block-ones [2, 128] bf16 for the per-pair recip broadcast MM:
            # row 0 -> cols 0:64 (head A), row 1 -> cols 64:128 (head B).
            # Partition-1 content is placed via SBUF->SBUF DMA (engine APs
            # cannot address base partition 1, DMA can).
            bo_f = cpool.tile([2, 128], F32)
            nc.vector.memset(bo_f[:, :], 0.0)
            nc.sync.dma_start(out=bo_f[0:1, 0:64], in_=ones_f32[0:1, 0:64])
            nc.sync.dma_start(out=bo_f[1:2, 64:128], in_=ones_f32[0:1, 0:64])
            blockones = cpool.tile([2, 128], BF16)
            nc.vector.tensor_copy(blockones[:, :], bo_f[:, :])

            # ---- SBUF tensors ----
            x_sb = xpool.tile([128, KT, N], BF16)
            wqk_sb = wpool.tile([128, KT, 2 * LQK], BF16)
            wv_sb = wpool.tile([128, KT, LQK], BF16)
            wproj_sb = wpool.tile([128, 2, D], BF16)
            # qk_sb m-tiles: m=0: q heads 0,1 / m=1: q heads 2,3
            #                m=2: k heads 0,1 / m=3: k heads 2,3
            qk_sb = qkpool.tile([128, 4, N], BF16)
            v_sb = vpool.tile([128, NT, HPC, HD], BF16)
            ao_sb = aopool.tile([128, 2, N], BF16)  # proj lhsT: kt2=pair

            # ---- DMAs, ordered so chain 0 can chase the x transfer ----
            # wqk k01 block (cols 256:384), q01 (0:128), wv; then x nb-major
            nc.sync.dma_start(out=wqk_sb[:, :, 256:384], in_=wqkT_r[:, :, 256:384])
            nc.sync.dma_start(out=wqk_sb[:, :, 0:128], in_=wqkT_r[:, :, 0:128])
            nc.sync.dma_start(out=wv_sb[:, :, :], in_=wvT_r[:, :, :])
            for nb in range(NB):
                for kt in range(KT):
                    nc.sync.dma_start(
                        out=x_sb[:, kt, nb * 512:(nb + 1) * 512],
                        in_=xT_r[:, kt, nb * 512:(nb + 1) * 512])
            nc.sync.dma_start(out=wqk_sb[:, :, 384:512], in_=wqkT_r[:, :, 384:512])
            nc.sync.dma_start(out=wqk_sb[:, :, 128:256], in_=wqkT_r[:, :, 128:256])
            nc.sync.dma_start(out=wproj_sb[:, :, :], in_=wprojT_r[:, :, :])

            # ---- gemm / proj work units (emitted inline or as fillers) ----
            def qk_m_nb(m, nb):
                """One [128,512] block of the q/k gemm: 8 kt MMs + copy."""
                wm = 0 if m < 2 else LQK
                wcol = wm + (m % 2) * 128
                ps = psprj.tile([128, 512], F32, tag="prj")
                for kt in range(KT):
                    nc.tensor.matmul(
                        ps[:, :],
                        wqk_sb[:, kt, wcol:wcol + 128],
                        x_sb[:, kt, nb * 512:(nb + 1) * 512],
                        start=(kt == 0), stop=(kt == KT - 1),
                    )
                nc.vector.tensor_copy(
                    qk_sb[:, m, nb * 512:(nb + 1) * 512], ps[:, :])

            def v_st(st):
                """v natural [128 seq, 4 heads x 64] for one seq tile."""
                ps = psprj.tile([128, 512], F32, tag="prj")
                for kt in range(KT):
                    nc.tensor.matmul(
                        ps[:, 0:LQK],
                        x_sb[:, kt, st * 128:(st + 1) * 128],
                        wv_sb[:, kt, :],
                        start=(kt == 0), stop=(kt == KT - 1),
                    )
                nc.vector.tensor_copy(
                    v_sb[:, st, :, :],
                    ps[:, 0:LQK].rearrange("p (h d) -> p h d", h=HPC))

            def proj_nt(nt):
                """Projection partial for one [128, 1024] output tile + DMA."""
                outst = outpool.tile([128, D], BF16, tag="outst")
                for ob in range(2):
                    ps = psprj.tile([128, 512], F32, tag="prj")
                    for kt2 in range(2):
                        nc.tensor.matmul(
                            ps[:, :],
                            ao_sb[:, kt2, nt * 128:(nt + 1) * 128],
                            wproj_sb[:, kt2, ob * 512:(ob + 1) * 512],
                            start=(kt2 == 0), stop=(kt2 == 1),
                        )
                    nc.vector.tensor_copy(
                        outst[:, ob * 512:(ob + 1) * 512], ps[:, :])
                nc.sync.dma_start(
                    out=out_d[nt * 128:(nt + 1) * 128, :], in_=outst[:, :])

            # soft filler queue (proj units; no ordering hazards vs chains
            # because each proj(qb) is appended only after both its pairs'
            # ao writes are emitted)
            fillers = []

            # ---- one attention chain: head pair p, q-block qb ----
            # `forced` units are data producers needed by LATER chains; they
            # are spread evenly across this chain's kt steps so emission
            # (and thus scheduler priority/dependency order) stays correct.
            def chain(p, qb, forced=(), interval=None):
                mq, mk = p, 2 + p
                forced = list(forced)
                if interval is None:
                    interval = max(1, NT // max(1, len(forced) + 1))
                pv = pspv.tile([128, 512], F32, tag="pv")
                den = psden.tile([128, 512], F32, tag="den")
                budget = 0.0
                last_sc = last_pr = None
                for kt in range(NT):
                    sc = pssc.tile([128, 1024], F32, tag="sc")
                    for i, pi in enumerate((0, 64)):
                        nc.tensor.matmul(
                            sc[:, i * 512:(i + 1) * 512],
                            qk_sb[pi:pi + 64, mk, kt * 128:(kt + 1) * 128],
                            qk_sb[pi:pi + 64, mq, qb * 512:(qb + 1) * 512],
                            start=True, stop=True,
                        )
                    pr = prpool.tile([128, 1024], BF16, tag="probs")
                    nc.scalar.activation(pr[:, :], sc[:, :], Exp, scale=SCALE)
                    if kt == NT - 1:
                        last_sc, last_pr = sc, pr
                    for i in range(2):
                        nc.tensor.matmul(
                            pv[i * 64:(i + 1) * 64, :],
                            v_sb[:, kt, 2 * p + i, :],
                            pr[:, i * 512:(i + 1) * 512],
                            start=(kt == 0), stop=(kt == NT - 1),
                            skip_group_check=True,
                        )
                    for i in range(2):
                        nc.tensor.matmul(
                            den[64 * i:64 * i + 1, :],
                            ones_col[:, :],
                            pr[:, i * 512:(i + 1) * 512],
                            start=(kt == 0), stop=(kt == NT - 1),
                            skip_group_check=True,
                        )
                    if forced and kt % interval == 0:
                        forced.pop(0)()
                    else:
                        budget += 0.45
                        while fillers and fillers[0][0] <= budget:
                            cost, fn = fillers.pop(0)
                            fn()
                            budget -= cost
                while forced:
                    forced.pop(0)()
                # ---- normalize tail ----
                # stage both denominators onto partition 0 (custom-DVE recip
                # and the broadcast MM rhs require base partition 0; the
                # partition remap 64->0 is done by SBUF->SBUF DMA)
                dsb = smpool.tile([65, 512], F32, tag="dsb")
                nc.vector.tensor_copy(dsb[64:65, :], den[64:65, :])
                dpair = smpool.tile([1, 1024], F32, tag="dpair")
                nc.vector.tensor_copy(dpair[0:1, 0:512], den[0:1, :])
                nc.sync.dma_start(out=dpair[0:1, 512:1024], in_=dsb[64:65, :])
                recip = smpool.tile([1, 1024], F32, tag="recip")
                nc.vector.reciprocal_approx_fast(recip[0:1, :], dpair[0:1, :])
                # head A recip -> partition 0, head B -> partition 1 (DMA)
                r2 = smpool.tile([2, 512], F32, tag="r2")
                nc.sync.dma_start(out=r2[0:1, :], in_=recip[0:1, 0:512])
                nc.sync.dma_start(out=r2[1:2, :], in_=recip[0:1, 512:1024])
                r2b = smpool.tile([2, 512], BF16, tag="r2b")
                nc.vector.tensor_copy(r2b[:, :], r2[:, :])
                bc = psprj.tile([128, 512], F32, tag="prj")
                nc.tensor.matmul(bc[:, :], blockones[:, :], r2b[:, :],
                                 start=True, stop=True)
                bcs = smpool.tile([128, 512], F32, tag="bcs")
                nc.vector.tensor_copy(bcs[:, :], bc[:, :])
                if debug and p == 0 and qb == 0:
                    dsc = smpool.tile([128, 1024], F32, tag="dbgsc")
                    nc.vector.tensor_copy(dsc[:, :], last_sc[:, :])
                    nc.sync.dma_start(out=dbg_sc_d[:, :], in_=dsc[:, :])
                    nc.sync.dma_start(out=dbg_pr_d[:, :], in_=last_pr[:, :])
                    dcp = smpool.tile([128, 512], F32, tag="dbgden")
                    nc.vector.memset(dcp[:, :], 0.0)
                    nc.vector.tensor_copy(dcp[0:1, :], den[0:1, :])
                    nc.vector.tensor_copy(dcp[64:65, :], den[64:65, :])
                    nc.sync.dma_start(out=dbg_den_d[:, :], in_=dcp[:, :])
                    dcp2 = smpool.tile([128, 512], F32, tag="dbgpv")
                    nc.vector.tensor_copy(dcp2[:, :], pv[:, :])
                    nc.sync.dma_start(out=dbg_pv_d[:, :], in_=dcp2[:, :])
                    nc.sync.dma_start(out=dbg_recip_d[:, :], in_=recip[0:1, :])
                    nc.sync.dma_start(out=dbg_bcs_d[:, :], in_=bcs[:, :])
                nc.vector.tensor_mul(
                    ao_sb[:, p, qb * 512:(qb + 1) * 512], pv[:, :], bcs[:, :])

            # ---- prologue: k01 (m=2) chasing the x DMA, q01 qb0, v st0/1 ----
            for nb in range(NB):
                qk_m_nb(2, nb)
            qk_m_nb(0, 0)
            v_st(0)
            v_st(1)

            def U_v(st):
                return lambda: v_st(st)

            def U_qk(m, nb):
                return lambda: qk_m_nb(m, nb)

            # forced (data-producing) fillers per chain, ordered by the
            # chain index that first consumes them:
            #   C1 needs q01-nb1; C2 q01-nb2; C3 q01-nb3;
            #   C4 (pair 1, qb0) needs k23 (m=3) all nb + q23-nb0;
            #   C5..C7 need q23-nb1..3.  v st2..15 are needed by C0 itself
            #   (PV of kt consumes v st=kt, emitted 2 steps ahead).
            forced_per_chain = [
                [U_v(st) for st in range(2, NT)] + [U_qk(0, 1)],   # during C0
                [U_qk(3, 0), U_qk(3, 1), U_qk(3, 2), U_qk(0, 2)],  # during C1
                [U_qk(3, 3), U_qk(1, 0), U_qk(0, 3)],              # during C2
                [U_qk(1, 1), U_qk(1, 2)],                          # during C3
                [U_qk(1, 3)],                                      # during C4
                [], [], [],
            ]

            # ---- chains, pair-minor: qb0..3 of pair 0, then pair 1 ----
            for ci in range(8):
                p, qb = ci // 4, ci % 4
                chain(p, qb, forced=forced_per_chain[ci],
                      interval=1 if ci == 0 else None)
                if ci >= 4:
                    for nt in range((ci - 4) * 4, (ci - 4) * 4 + 4):
                        fillers.append(
                            (0.9, (lambda t: lambda: proj_nt(t))(nt)))
            # drain remaining fillers (proj of qb3 + any leftovers)
            while fillers:
                fillers.pop(0)[1]()

            if debug:
                nc.sync.dma_start(
                    out=dbg_qk_d[:, :],
                    in_=qk_sb[:, :, :].rearrange("p a b -> p (a b)"))
                nc.sync.dma_start(
                    out=dbg_v_d[:, :],
                    in_=v_sb[:, :, :, :].rearrange("p a b c -> p (a b c)"))
                nc.sync.dma_start(
                    out=dbg_ao_d[:, :],
                    in_=ao_sb[:, :, :].rearrange("p a b -> p (a b)"))

    nc.compile()
    return nc


def _get_program():
    if "nc" not in _CACHE:
        _CACHE["nc"] = _build_program()
    return _CACHE["nc"]


def _make_in_maps(x, w_qkv, w_proj):
    import ml_dtypes
    bf16 = ml_dtypes.bfloat16
    x = np.asarray(x, dtype=np.float32)
    w_qkv = np.asarray(w_qkv, dtype=np.float32)
    w_proj = np.asarray(w_proj, dtype=np.float32)
    xT = [np.ascontiguousarray(x[b].T).astype(bf16) for b in range(B)]
    in_maps = []
    for c in range(NCORES):
        b, hg = c // 4, c % 4
        rows = slice(hg * LQK, (hg + 1) * LQK)
        qk_rows = np.r_[np.arange(hg * LQK, (hg + 1) * LQK),
                        D + np.arange(hg * LQK, (hg + 1) * LQK)]
        in_maps.append({
            "xT": xT[b],
            "wqkT": np.ascontiguousarray(w_qkv[qk_rows, :].T).astype(bf16),
            "wvT": np.ascontiguousarray(
                w_qkv[2 * D + np.arange(hg * LQK, (hg + 1) * LQK), :].T).astype(bf16),
            "wprojT": np.ascontiguousarray(w_proj[:, rows].T).astype(bf16),
        })
    return in_maps


def kernel(x, w_qkv, w_proj, b_proj, _return_results=False, _trace=False):
    from concourse import bass_utils

    nc = _get_program()
    in_maps = _make_in_maps(x, w_qkv, w_proj)
    res = bass_utils.run_bass_kernel_spmd(
        nc, in_maps, list(range(NCORES)), trace=_trace)
    partials = np.stack(
        [np.asarray(res.results[c]["out"], dtype=np.float32)
         for c in range(NCORES)])
    out = partials.reshape(B, 4, N, D).sum(axis=1, dtype=np.float32)
    out = out + np.asarray(b_proj, dtype=np.float32)[None, None, :]
    out = out.astype(np.float32)
    if _return_results:
        return out, res
    return out


# revision 33
# speedup vs baseline: 1.1388x; 1.1388x over previous
"""Multi-head attention (B=2, N=2048, D=1024, H=16, hd=64) on 8 trn2 NeuronCores.

Sharding: 8 cores = 2 (batch) x 4 (head groups of 4 heads).
Core c: batch b = c // 4, heads hg*4 .. hg*4+3 where hg = c % 4.

Per-core program (identical SPMD program, per-core data). All inputs are
repacked on the host into the exact SBUF layout ([128 partitions, ...]
with >=2KB contiguous per partition line) so input DMAs run at full HBM
bandwidth:
  xr     [128, NB*KT*512]  x[b].T as [p][nb][kt][512]
  wqkr   [128, 4*KT*128]   w_qkv q/k rows as [p][m][kt][128]
                           (m: 0=q heads01, 1=q heads23, 2=k h01, 3=k h23)
  wvr    [128, KT*256]     w_qkv v rows as [p][kt][256]
  wprojr [128, 2*1024]     w_proj local cols as [p][kt2][1024]
  out    [2048, 1024]      bf16 partial (row-parallel) projection output

Attention runs as 8 ACT-paced chains (pair-minor: 4 q-blocks of head
pair (0,1), then of pair (2,3)).  Per chain, per key tile kt:
  - 2 score MMs (heads at PE row groups 0/64 -> concurrent), one
    [128,1024] sc PSUM tile, double-buffered against the ACT
  - 1 ACT exp over both heads' scores -> probs bf16 in SBUF
  - 2 PV MMs col-packed (M=64 at col groups 0/64) into one pv bank
  - 2 denominator MMs (M=2 ones/zeros lhsT -> heads land on PSUM
    partitions 0/1 of the den bank, no partition shuffling needed)
The normalize tail (recip_approx + block-ones broadcast MM + DVE mul
into ao_sb) is EMITTED a few kt-steps into the next chain so its
cross-engine latency never blocks the PE queue head.  qkv gemms, the v
gemm, projection and output DMA are emitted as paced filler units
inside the chains to fill the PE's slack under the ACT.

Host unshard: out[b] = sum over 4 head-group partials + b_proj.
"""

import sys

if "/opt/trn_rl_repo" not in sys.path:
    sys.path.insert(0, "/opt/trn_rl_repo")

import numpy as np

B, N, D, H, HD = 2, 2048, 1024, 16, 64
NCORES = 8
HPC = 4               # heads per core
LQK = HPC * HD        # 256 local q (or k) rows
SCALE = HD ** -0.5    # 0.125

_CACHE = {}


def _build_program(debug=False):
    import concourse.tile as tile
    from concourse import bacc, mybir

    F32 = mybir.dt.float32
    BF16 = mybir.dt.bfloat16
    Exp = mybir.ActivationFunctionType.Exp

    nc = bacc.Bacc("TRN2", target_bir_lowering=False, debug=False,
                   num_devices=NCORES)

    KT = D // 128        # 8 contraction tiles for qkv gemms
    NB = N // 512        # 4 seq blocks
    NT = N // 128        # 16 seq tiles

    xr_d = nc.dram_tensor("xr", [128, NB * KT * 512], BF16,
                          kind="ExternalInput").ap()
    wqkr_d = nc.dram_tensor("wqkr", [128, 4 * KT * 128], BF16,
                            kind="ExternalInput").ap()
    wvr_d = nc.dram_tensor("wvr", [128, KT * LQK], BF16,
                           kind="ExternalInput").ap()
    wprojr_d = nc.dram_tensor("wprojr", [128, 2 * D], BF16,
                              kind="ExternalInput").ap()
    out_d = nc.dram_tensor("out", [N, D], BF16, kind="ExternalOutput").ap()
    if debug:
        dbg_qk_d = nc.dram_tensor("dbg_qk", [128, 4 * N], BF16,
                                  kind="ExternalOutput").ap()
        dbg_v_d = nc.dram_tensor("dbg_v", [128, NT * HPC * HD], BF16,
                                 kind="ExternalOutput").ap()
        dbg_ao_d = nc.dram_tensor("dbg_ao", [128, 2 * N], BF16,
                                  kind="ExternalOutput").ap()
        dbg_den_d = nc.dram_tensor("dbg_den", [2, 512], mybir.dt.float32,
                                   kind="ExternalOutput").ap()
        dbg_pv_d = nc.dram_tensor("dbg_pv", [128, 512], mybir.dt.float32,
                                  kind="ExternalOutput").ap()
        dbg_recip_d = nc.dram_tensor("dbg_recip", [2, 512], mybir.dt.float32,
                                     kind="ExternalOutput").ap()
        dbg_bcs_d = nc.dram_tensor("dbg_bcs", [128, 512], mybir.dt.float32,
                                   kind="ExternalOutput").ap()

    xr_r = xr_d.rearrange("p (nb kt c) -> p nb kt c", nb=NB, kt=KT)
    wqkr_r = wqkr_d.rearrange("p (m kt c) -> p m kt c", m=4, kt=KT)
    wvr_r = wvr_d.rearrange("p (kt c) -> p kt c", kt=KT)
    wprojr_r = wprojr_d.rearrange("p (k c) -> p k c", k=2)

    with tile.TileContext(nc) as tc:
        with (
            nc.allow_low_precision(reason="bf16 matmul operands"),
            tc.tile_pool(name="const", bufs=1) as cpool,
            tc.tile_pool(name="w", bufs=1) as wpool,
            tc.tile_pool(name="x", bufs=1) as xpool,
            tc.tile_pool(name="qk", bufs=1) as qkpool,
            tc.tile_pool(name="v", bufs=1) as vpool,
            tc.tile_pool(name="ao", bufs=1) as aopool,
            tc.tile_pool(name="probs", bufs=3) as prpool,
            tc.tile_pool(name="small", bufs=2) as smpool,
            tc.tile_pool(name="outs", bufs=2) as outpool,
            tc.tile_pool(name="pssc", bufs=2, space="PSUM") as pssc,
            tc.tile_pool(name="pspv", bufs=1, space="PSUM") as pspv,
            tc.tile_pool(name="psden", bufs=1, space="PSUM") as psden,
            tc.tile_pool(name="psprj", bufs=2, space="PSUM") as psprj,
        ):
            # ---- constants ----
            ones_f32 = cpool.tile([128, 128], F32)
            nc.vector.memset(ones_f32[:, :], 1.0)
            # ones/zeros column pairs: denominator MMs with M=2 so head A
            # lands on den partition 0 and head B on partition 1 (dst base
            # stays 0 -> no small-tile dst-partition ISA restrictions)
            oz = cpool.tile([128, 2], BF16)
            nc.vector.memset(oz[:, 0:1], 1.0)
            nc.vector.memset(oz[:, 1:2], 0.0)
            zo = cpool.tile([128, 2], BF16)
            nc.vector.memset(zo[:, 0:1], 0.0)
            nc.vector.memset(zo[:, 1:2], 1.0)
            # block-ones [2, 128] for the per-pair recip broadcast MM:
            # row 0 -> cols 0:64 (head A), row 1 -> cols 64:128 (head B).
            # Partition-1 content is placed via SBUF->SBUF DMA (engine APs
            # cannot address base partition 1, DMA can).
            bo_f = cpool.tile([2, 128], F32)
            nc.vector.memset(bo_f[:, :], 0.0)
            nc.sync.dma_start(out=bo_f[0:1, 0:64], in_=ones_f32[0:1, 0:64])
            nc.sync.dma_start(out=bo_f[1:2, 64:128], in_=ones_f32[0:1, 0:64])
            blockones = cpool.tile([2, 128], BF16)
            nc.vector.tensor_copy(blockones[:, :], bo_f[:, :])

            # ---- SBUF tensors ----
            x_sb = xpool.tile([128, NB, KT, 512], BF16)
            wqk_sb = wpool.tile([128, 4, KT, 128], BF16)
            wv_sb = wpool.tile([128, KT, LQK], BF16)
            wproj_sb = wpool.tile([128, 2, D], BF16)
            # qk_sb m-tiles: m=0: q heads 0,1 / m=1: q heads 2,3
            #                m=2: k heads 0,1 / m=3: k heads 2,3
            qk_sb = qkpool.tile([128, 4, N], BF16)
            v_sb = vpool.tile([128, NT, HPC, HD], BF16)
            ao_sb = aopool.tile([128, 2, N], BF16)  # proj lhsT: kt2=pair

            # ---- DMAs: big contiguous chunks, ordered for chain 0 chase ----
            nc.sync.dma_start(out=wqk_sb[:, 2, :, :], in_=wqkr_r[:, 2, :, :])
            nc.sync.dma_start(out=wqk_sb[:, 0, :, :], in_=wqkr_r[:, 0, :, :])
            nc.sync.dma_start(out=wv_sb[:, :, :], in_=wvr_r[:, :, :])
            for nb in range(NB):
                nc.sync.dma_start(out=x_sb[:, nb, :, :], in_=xr_r[:, nb, :, :])
            nc.sync.dma_start(out=wqk_sb[:, 3, :, :], in_=wqkr_r[:, 3, :, :])
            nc.sync.dma_start(out=wqk_sb[:, 1, :, :], in_=wqkr_r[:, 1, :, :])
            nc.sync.dma_start(out=wproj_sb[:, :, :], in_=wprojr_r[:, :, :])

            # ---- filler work units (each ~0.5-1.0us of PE time) ----
            def qk_m_nb(m, nb):
                """q/k gemm [128,512] block as 2 units sharing one psum."""
                state = {}

                def half(first):
                    if first:
                        ps = psprj.tile([128, 512], F32, tag="prj")
                        state["ps"] = ps
                    ps = state["ps"]
                    for kt in (range(0, 4) if first else range(4, KT)):
                        nc.tensor.matmul(
                            ps[:, :],
                            wqk_sb[:, m, kt, :],
                            x_sb[:, nb, kt, :],
                            start=(kt == 0), stop=(kt == KT - 1),
                        )
                    if not first:
                        nc.vector.tensor_copy(
                            qk_sb[:, m, nb * 512:(nb + 1) * 512], ps[:, :])

                return [(0.95, lambda: half(True)), (0.95, lambda: half(False))]

            def v_st(st):
                """v natural [128 seq, 4x64] for one seq tile, 2 units."""
                state = {}
                nb, c = st // 4, (st % 4) * 128

                def half(first):
                    if first:
                        ps = psprj.tile([128, 512], F32, tag="prj")
                        state["ps"] = ps
                    ps = state["ps"]
                    for kt in (range(0, 4) if first else range(4, KT)):
                        nc.tensor.matmul(
                            ps[:, 0:LQK],
                            x_sb[:, nb, kt, c:c + 128],
                            wv_sb[:, kt, :],
                            start=(kt == 0), stop=(kt == KT - 1),
                        )
                    if not first:
                        nc.vector.tensor_copy(
                            v_sb[:, st, :, :],
                            ps[:, 0:LQK].rearrange("p (h d) -> p h d", h=HPC))

                return [(0.5, lambda: half(True)), (0.5, lambda: half(False))]

            def proj_nt(nt):
                """Projection partial for one [128, 1024] output tile + DMA."""
                outst = outpool.tile([128, D], BF16, tag="outst")
                for ob in range(2):
                    ps = psprj.tile([128, 512], F32, tag="prj")
                    for kt2 in range(2):
                        nc.tensor.matmul(
                            ps[:, :],
                            ao_sb[:, kt2, nt * 128:(nt + 1) * 128],
                            wproj_sb[:, kt2, ob * 512:(ob + 1) * 512],
                            start=(kt2 == 0), stop=(kt2 == 1),
                        )
                    nc.vector.tensor_copy(
                        outst[:, ob * 512:(ob + 1) * 512], ps[:, :])
                nc.sync.dma_start(
                    out=out_d[nt * 128:(nt + 1) * 128, :], in_=outst[:, :])

            fillers = []

            # ---- one attention chain: head pair p, q-block qb ----
            def chain(p, qb, forced=(), prev_tail=None):
                mq, mk = p, 2 + p
                forced = list(forced)
                pv = pspv.tile([128, 512], F32, tag="pv")
                den = psden.tile([128, 512], F32, tag="den")
                budget = 0.0
                for kt in range(NT):
                    sc = pssc.tile([128, 1024], F32, tag="sc")
                    for i, pi in enumerate((0, 64)):
                        nc.tensor.matmul(
                            sc[:, i * 512:(i + 1) * 512],
                            qk_sb[pi:pi + 64, mk, kt * 128:(kt + 1) * 128],
                            qk_sb[pi:pi + 64, mq, qb * 512:(qb + 1) * 512],
                            start=True, stop=True,
                        )
                    pr = prpool.tile([128, 1024], BF16, tag="probs")
                    nc.scalar.activation(pr[:, :], sc[:, :], Exp, scale=SCALE)
                    for i in range(2):
                        nc.tensor.matmul(
                            pv[i * 64:(i + 1) * 64, :],
                            v_sb[:, kt, 2 * p + i, :],
                            pr[:, i * 512:(i + 1) * 512],
                            start=(kt == 0), stop=(kt == NT - 1),
                            skip_group_check=True,
                        )
                    for i in range(2):
                        nc.tensor.matmul(
                            den[0:2, :],
                            (oz if i == 0 else zo)[:, :],
                            pr[:, i * 512:(i + 1) * 512],
                            start=(kt == 0 and i == 0),
                            stop=(kt == NT - 1 and i == 1),
                            skip_group_check=True,
                        )
                    # previous chain's normalize tail, off the queue head
                    if kt == 2 and prev_tail is not None:
                        prev_tail()
                    # forced data-producing fillers (needed by later chains)
                    npop = -(-len(forced) // (NT - kt)) if forced else 0
                    for _ in range(npop):
                        forced.pop(0)()
                    if not npop and kt >= 3:
                        budget += 0.45
                        while fillers and fillers[0][0] <= budget:
                            cost, fn = fillers.pop(0)
                            fn()
                            budget -= cost

                def tail():
                    d2 = smpool.tile([2, 512], F32, tag="d2")
                    nc.vector.tensor_copy(d2[:, :], den[0:2, :])
                    recip = smpool.tile([2, 512], F32, tag="recip")
                    nc.vector.reciprocal_approx_fast(recip[:, :], d2[:, :])
                    r2b = smpool.tile([2, 512], BF16, tag="r2b")
                    nc.vector.tensor_copy(r2b[:, :], recip[:, :])
                    bc = psprj.tile([128, 512], F32, tag="prj")
                    nc.tensor.matmul(bc[:, :], blockones[:, :], r2b[:, :],
                                     start=True, stop=True)
                    bcs = smpool.tile([128, 512], F32, tag="bcs")
                    nc.vector.tensor_copy(bcs[:, :], bc[:, :])
                    if debug and p == 0 and qb == 0:
                        nc.sync.dma_start(out=dbg_den_d[:, :], in_=d2[:, :])
                        dcp2 = smpool.tile([128, 512], F32, tag="dbgpv")
                        nc.vector.tensor_copy(dcp2[:, :], pv[:, :])
                        nc.sync.dma_start(out=dbg_pv_d[:, :], in_=dcp2[:, :])
                        nc.sync.dma_start(out=dbg_recip_d[:, :],
                                          in_=recip[:, :])
                        nc.sync.dma_start(out=dbg_bcs_d[:, :], in_=bcs[:, :])
                    nc.vector.tensor_mul(
                        ao_sb[:, p, qb * 512:(qb + 1) * 512],
                        pv[:, :], bcs[:, :])

                return tail

            # ---- prologue: just enough for chain 0 to start chasing ----
            for u in qk_m_nb(2, 0) + qk_m_nb(0, 0) + v_st(0) + v_st(1):
                u[1]()

            def U(units):
                return [u[1] for u in units]

            # forced fillers per chain, ordered by first-consumer chain.
            # Chain 0 consumes k01/v progressively (scores kt needs m2-nb
            # kt//4, PV kt needs v st kt); later chains' q-blocks and pair-1
            # weights are produced 1+ chains ahead of use.
            forced_per_chain = [
                # during C0 (pair0 qb0): m2 nb1-3 + v st2..15 interleaved by
                # need time, then q01-nb1 (needed by C1)
                U(qk_m_nb(2, 1)) + U(v_st(2)) + U(v_st(3))
                + U(v_st(4)) + U(qk_m_nb(2, 2)) + U(v_st(5)) + U(v_st(6))
                + U(v_st(7)) + U(qk_m_nb(2, 3)) + U(v_st(8)) + U(v_st(9))
                + U(v_st(10)) + U(v_st(11)) + U(v_st(12)) + U(v_st(13))
                + U(v_st(14)) + U(v_st(15)) + U(qk_m_nb(0, 1)),
                U(qk_m_nb(3, 0)) + U(qk_m_nb(3, 1)) + U(qk_m_nb(0, 2)),
                U(qk_m_nb(3, 2)) + U(qk_m_nb(3, 3)) + U(qk_m_nb(0, 3)),
                U(qk_m_nb(1, 0)) + U(qk_m_nb(1, 1)),
                U(qk_m_nb(1, 2)) + U(qk_m_nb(1, 3)),
                [], [], [],
            ]

            # ---- chains, pair-minor: qb0..3 of pair 0, then pair 1 ----
            prev_tail = None
            for ci in range(8):
                p, qb = ci // 4, ci % 4
                prev_tail = chain(p, qb, forced=forced_per_chain[ci],
                                  prev_tail=prev_tail)
                if ci >= 4:
                    for nt in range((ci - 4) * 4, (ci - 4) * 4 + 4):
                        fillers.append(
                            (0.9, (lambda t: lambda: proj_nt(t))(nt)))
            prev_tail()
            while fillers:
                fillers.pop(0)[1]()

            if debug:
                nc.sync.dma_start(
                    out=dbg_qk_d[:, :],
                    in_=qk_sb[:, :, :].rearrange("p a b -> p (a b)"))
                nc.sync.dma_start(
                    out=dbg_v_d[:, :],
                    in_=v_sb[:, :, :, :].rearrange("p a b c -> p (a b c)"))
                nc.sync.dma_start(
                    out=dbg_ao_d[:, :],
                    in_=ao_sb[:, :, :].rearrange("p a b -> p (a b)"))

    nc.compile()
    return nc


def _get_program():
    if "nc" not in _CACHE:
        _CACHE["nc"] = _build_program()
    return _CACHE["nc"]


def _make_in_maps(x, w_qkv, w_proj):
    import ml_dtypes
    bf16 = ml_dtypes.bfloat16
    x = np.asarray(x, dtype=np.float32)
    w_qkv = np.asarray(w_qkv, dtype=np.float32)
    w_proj = np.asarray(w_proj, dtype=np.float32)
    KT, NB = D // 128, N // 512
    # x[b].T reshaped to SBUF layout [p][nb][kt][512]
    xr = []
    for b in range(B):
        t = x[b].T.reshape(KT, 128, NB, 512)          # [kt, p, nb, c]
        xr.append(np.ascontiguousarray(
            t.transpose(1, 2, 0, 3).reshape(128, NB * KT * 512)).astype(bf16))
    in_maps = []
    for c in range(NCORES):
        b, hg = c // 4, c % 4
        rows = slice(hg * LQK, (hg + 1) * LQK)
        # m blocks: q h01, q h23, k h01, k h23 of this head group
        qoff = hg * LQK
        koff = D + hg * LQK
        mrows = [np.arange(qoff, qoff + 128),
                 np.arange(qoff + 128, qoff + 256),
                 np.arange(koff, koff + 128),
                 np.arange(koff + 128, koff + 256)]
        # wqkT [p][m][kt][128]: block m, contraction tile kt ->
        # w_qkv[mrows[m]][kt*128+p] transposed
        wq = np.stack([w_qkv[r, :].T.reshape(KT, 128, 128) for r in mrows])
        wqkr = np.ascontiguousarray(
            wq.transpose(2, 0, 1, 3).reshape(128, 4 * KT * 128)).astype(bf16)
        wvt = w_qkv[2 * D + np.arange(hg * LQK, (hg + 1) * LQK), :].T
        wvr = np.ascontiguousarray(
            wvt.reshape(KT, 128, LQK).transpose(1, 0, 2).reshape(
                128, KT * LQK)).astype(bf16)
        wpt = w_proj[:, rows].T                        # [256 local, 1024]
        wprojr = np.ascontiguousarray(
            wpt.reshape(2, 128, D).transpose(1, 0, 2).reshape(
                128, 2 * D)).astype(bf16)
        in_maps.append({
            "xr": xr[b],
            "wqkr": wqkr,
            "wvr": wvr,
            "wprojr": wprojr,
        })
    return in_maps


def kernel(x, w_qkv, w_proj, b_proj, _return_results=False, _trace=False):
    from concourse import bass_utils

    nc = _get_program()
    in_maps = _make_in_maps(x, w_qkv, w_proj)
    res = bass_utils.run_bass_kernel_spmd(
        nc, in_maps, list(range(NCORES)), trace=_trace)
    partials = np.stack(
        [np.asarray(res.results[c]["out"], dtype=np.float32)
         for c in range(NCORES)])
    out = partials.reshape(B, 4, N, D).sum(axis=1, dtype=np.float32)
    out = out + np.asarray(b_proj, dtype=np.float32)[None, None, :]
    out = out.astype(np.float32)
    if _return_results:
        return out, res
    return out
